# revision 9
# baseline (speedup 1.0000x reference)
"""MoE (top-1 routing, E=8 experts) Trainium2 kernel.

Strategy (expert-parallel across 8 NeuronCores):
  - Routing (softmax/argmax/capacity) is computed on host with jax-on-CPU,
    replicating the reference computation op-for-op so expert assignment
    matches bit-exactly.
  - Dispatch (the "all-to-all") happens host-side while building per-core
    inputs: core e receives the (<=2048) tokens routed to expert e, already
    gathered, scaled by gate probability, transposed to [D, cap], and cast
    to fp16 (same PE rate as fp32r, half the DMA bytes, FWL weight loads).
  - Each core runs Y_e = relu(Xe @ W1_e) @ W2_e as a dense FFN with all
    weights + tokens resident in SBUF. GEMM2 partials per F-block are
    evacuated as fp16 and summed on host (frees SBUF + vector engine,
    streams the output DMA throughout the kernel instead of a tail burst).
  - Combine: host sums the 8 F-block partials, scatters back to token order.
"""

import os
import sys

for _p in ("/opt/trn_rl_repo",):
    if os.path.isdir(_p) and _p not in sys.path:
        sys.path.insert(0, _p)

import numpy as np

B, S, D, F, E = 8, 2048, 1024, 4096, 8
T = B * S
CAP = T // E  # 2048, capacity_factor 1.0

F_BLK = 512          # F columns per outer block
N_FBLK = F // F_BLK  # 8
N_DC = D // 128      # 8 contraction chunks for GEMM1
N_FC = F_BLK // 128  # 4 contraction chunks for GEMM2 per block
N_TG = CAP // 128    # 16 token groups
N_TC = CAP // 512    # 4 token columns


def _build_nc():
    import concourse.bacc as bacc
    import concourse.mybir as mybir
    from concourse.bass import ds
    from concourse.tile import TileContext

    f32 = mybir.dt.float32
    f16 = mybir.dt.float16
    f8 = mybir.dt.float8e4

    nc = bacc.Bacc("TRN2", target_bir_lowering=False, debug=False, num_devices=E)

    # Host pre-tiles the inputs so every DMA line is >=4 KB contiguous per
    # partition: w1 [fo, p, dc, 512], w2 [fo, p, fc, 1024], xeT [c, p, dc, 256].
    xeT = nc.dram_tensor("xeT", [8, 128, N_DC, 256], f16, kind="ExternalInput")
    w1 = nc.dram_tensor("w1", [N_FBLK, 128, N_DC, F_BLK], f16, kind="ExternalInput")
    w2 = nc.dram_tensor("w2", [N_FBLK, 128, N_FC, D], f16, kind="ExternalInput")
    # fo=0's W2 block, scaled by 1024 and cast to e4m3 for DoubleRow matmuls.
    w28 = nc.dram_tensor("w28", [128, N_FC, D], f8, kind="ExternalInput")
    # Per-F-block GEMM2 partials; host sums over axis 0.
    y = nc.dram_tensor("y", [N_FBLK, CAP, D], f16, kind="ExternalOutput")

    x_r = xeT.ap().rearrange("c p dc j -> p c dc j")
    w1_r = w1.ap().rearrange("fo p dc j -> p fo dc j")
    w2_r = w2.ap().rearrange("fo p fc d -> p fo fc d")
    y_r = y.ap().rearrange("fo (tg p) d -> p fo tg d", p=128)

    with TileContext(nc) as tc:
        with (
            tc.tile_pool(name="sbuf", bufs=1) as sbuf,
            tc.tile_pool(name="spool", bufs=2) as spool,
            tc.tile_pool(name="psh", bufs=3, space="PSUM") as psh,
            tc.tile_pool(name="psy", bufs=5, space="PSUM") as psy,
        ):
            # PE warmup: dependency-light fp32 matmuls ramp the HAM clock to
            # 2.4 GHz while the first DMAs land. memset on the vector queue
            # starts earlier than gpsimd's.
            warm_sb = sbuf.tile([128, 384], f32, tag="warm")
            nc.vector.memset(warm_sb, 0)
            for _ in range(16):
                pwarm = psy.tile([128, 512], f32, tag="py")
                nc.tensor.matmul(
                    pwarm[:, :256], warm_sb[:, :128], warm_sb[:, ds(128, 256)],
                    start=True, stop=True,
                )

            # The gpsimd SWDGE queue moves data ~10x faster than the sync
            # HWDGE queue during the startup window, so everything the fo=0
            # block needs streams there, interleaved in consumption order:
            # w1[fo0], first xeT chunks, w2[fo0], remaining xeT chunks. The
            # bulk fo>=1 weights stream on sync in parallel (they are needed
            # only ~55us/block later).
            xeT_sb = sbuf.tile([128, N_DC, CAP], f16, tag="x")
            w1_sb = sbuf.tile([128, N_DC, F], f16, tag="w1")
            w2_sb = sbuf.tile([128, F // 128, D], f16, tag="w2")

            w28_sb = sbuf.tile([128, N_FC, D], f8, tag="w28")
            hT8 = sbuf.tile([128, N_FC, CAP], f8, tag="h8")

            # Both DMA queues ramp from ~70 GB/s at startup, so the critical
            # first-block bytes are split across them: w1[fo0] on sync, xeT
            # chunks on gpsimd. fo1 weights also go on gpsimd (sync is still
            # ramping when fo1's compute starts); the rest stream on sync.
            nc.sync.dma_start(out=w1_sb[:, :, ds(0, F_BLK)], in_=w1_r[:, 0, :, :])
            for tcix in range(3):
                nc.gpsimd.dma_start(
                    out=xeT_sb[:, :, ds(tcix * 256, 256)],
                    in_=x_r[:, tcix, :, :],
                )
            nc.gpsimd.dma_start(out=w28_sb, in_=w28.ap())
            for tcix in range(3, 8):
                nc.gpsimd.dma_start(
                    out=xeT_sb[:, :, ds(tcix * 256, 256)],
                    in_=x_r[:, tcix, :, :],
                )
            nc.gpsimd.dma_start(out=w1_sb[:, :, ds(F_BLK, F_BLK)], in_=w1_r[:, 1, :, :])
            nc.gpsimd.dma_start(out=w2_sb[:, ds(N_FC, N_FC), :], in_=w2_r[:, 1, :, :])

            for fo in range(2, N_FBLK):
                nc.sync.dma_start(
                    out=w1_sb[:, :, ds(fo * F_BLK, F_BLK)],
                    in_=w1_r[:, fo, :, :],
                )
                nc.sync.dma_start(
                    out=w2_sb[:, ds(fo * N_FC, N_FC), :],
                    in_=w2_r[:, fo, :, :],
                )

            hT = sbuf.tile([128, N_FC, CAP], f16, tag="h")

            for fo in range(N_FBLK):
                # GEMM1: hT[f, t] = relu(sum_d W1[d, f] * XeT[d, t])
                # fo==0 walks 256-token columns to match streaming xeT
                # arrival; later blocks use full 512-wide moving operands.
                tok_cols = 2 * N_TC if fo == 0 else N_TC
                tok_w = CAP // tok_cols
                for tcix in range(tok_cols):
                    for fc in range(N_FC):
                        ph = psh.tile([128, 512], f32, tag="ph")
                        for dc in range(N_DC):
                            nc.tensor.matmul(
                                ph[:, :tok_w],
                                w1_sb[:, dc, ds(fo * F_BLK + fc * 128, 128)],
                                xeT_sb[:, dc, ds(tcix * tok_w, tok_w)],
                                start=(dc == 0),
                                stop=(dc == N_DC - 1),
                            )
                        if fo == 0:
                            nc.scalar.activation(
                                hT8[:, fc, ds(tcix * tok_w, tok_w)],
                                ph[:, :tok_w],
                                mybir.ActivationFunctionType.Relu,
                                scale=16.0,
                            )
                        else:
                            nc.scalar.activation(
                                hT[:, fc, ds(tcix * tok_w, tok_w)],
                                ph[:, :tok_w],
                                mybir.ActivationFunctionType.Relu,
                            )

                # GEMM2: y_fo[t, d] = sum_f hT[f, t] * W2[f, d]; evacuate each
                # [128, 1024] token-group row as fp16 and stream it out on the
                # scalar (Activation) queue, summed across fo on host.
                for tg in range(N_TG):
                    if tg % 2 == 0:
                        stage = spool.tile([128, 2, D], f16, tag="st")
                    for dh in range(2):
                        py = psy.tile([128, 512], f32, tag="py")
                        if fo == 0:
                            # fp8 DoubleRow: contract f-chunk pairs (K=256 per
                            # matmul) at 2x rate; inputs carry a 16*1024 scale
                            # removed at evacuation.
                            for dq in range(2):
                                for pr in range(2):
                                    nc.tensor.matmul(
                                        py[:, ds(dq * 256, 256)],
                                        hT8[:, ds(2 * pr, 2), ds(tg * 128, 128)],
                                        w28_sb[:, ds(2 * pr, 2),
                                               ds(dh * 512 + dq * 256, 256)],
                                        start=(pr == 0),
                                        stop=(pr == 1),
                                        perf_mode=mybir.MatmulPerfMode.DoubleRow,
                                    )
                            nc.vector.tensor_scalar_mul(
                                stage[:, tg % 2, ds(dh * 512, 512)], py,
                                1.0 / 16384.0,
                            )
                        else:
                            for fc in range(N_FC):
                                nc.tensor.matmul(
                                    py,
                                    hT[:, fc, ds(tg * 128, 128)],
                                    w2_sb[:, fo * N_FC + fc, ds(dh * 512, 512)],
                                    start=(fc == 0),
                                    stop=(fc == N_FC - 1),
                                )
                            nc.vector.tensor_copy(stage[:, tg % 2, ds(dh * 512, 512)], py)
                    if tg % 2 == 1:
                        nc.scalar.dma_start(
                            out=y_r[:, fo, ds(tg - 1, 2), :],
                            in_=stage,
                        )

    nc.compile()
    return nc


_NC = None


def _get_nc():
    global _NC
    if _NC is None:
        _NC = _build_nc()
    return _NC


def _route(xf, Wr):
    """Replicates the reference routing (jax-on-CPU, op-for-op) so that
    expert assignment matches the fp32 reference bit-exactly."""
    try:
        import jax
        import jax.numpy as jnp

        cpu = jax.local_devices(backend="cpu")[0]
        with jax.default_device(cpu):
            xj = jnp.asarray(xf, dtype=jnp.float32)
            wj = jnp.asarray(Wr, dtype=jnp.float32)
            probs = jax.nn.softmax(xj @ wj, axis=-1)
            eidx_j = jnp.argmax(probs, axis=-1)
            p_tok_j = jnp.take_along_axis(probs, eidx_j[:, None], axis=1)[:, 0]
            eidx = np.asarray(eidx_j)
            p_tok = np.asarray(p_tok_j)
    except Exception:
        # numpy fallback (fp32, same math; argmax ties broken identically
        # by first-max)
        logits = xf.astype(np.float32) @ Wr.astype(np.float32)
        lmax = logits.max(axis=-1, keepdims=True)
        ex = np.exp(logits - lmax)
        probs = ex / ex.sum(axis=-1, keepdims=True)
        eidx = np.argmax(probs, axis=-1)
        p_tok = probs[np.arange(T), eidx]

    # Integer capacity logic (exact) in numpy.
    onehot = np.zeros((T, E), dtype=np.int64)
    onehot[np.arange(T), eidx] = 1
    rank = np.cumsum(onehot, axis=0) - onehot
    rank = rank[np.arange(T), eidx]  # earlier same-expert tokens
    keep = rank < CAP

    dispatch = np.zeros((E, CAP), dtype=np.int64)
    valid = np.zeros((E, CAP), dtype=bool)
    kept = np.nonzero(keep)[0]
    dispatch[eidx[kept], rank[kept]] = kept
    valid[eidx[kept], rank[kept]] = True
    return dispatch, valid, p_tok


def kernel(x, Wr, W1, W2):
    from concourse.bass_utils import run_bass_kernel_spmd

    x = np.asarray(x, dtype=np.float32)
    Wr = np.asarray(Wr, dtype=np.float32)
    W1 = np.asarray(W1, dtype=np.float32)
    W2 = np.asarray(W2, dtype=np.float32)

    xf = x.reshape(T, D)
    dispatch, valid, p_tok = _route(xf, Wr)

    in_maps = []
    for e in range(E):
        scale = np.where(valid[e], p_tok[dispatch[e]], 0.0).astype(np.float32)
        xe = xf[dispatch[e]] * scale[:, None]  # [CAP, D]; relu(s*x@W1)@W2 = s*y
        xeT_t = xe.T.astype(np.float16).reshape(N_DC, 128, 8, 256).transpose(2, 1, 0, 3)
        w1_t = W1[e].astype(np.float16).reshape(N_DC, 128, N_FBLK, F_BLK).transpose(2, 1, 0, 3)
        w2_t = W2[e].astype(np.float16).reshape(N_FBLK, N_FC, 128, D).transpose(0, 2, 1, 3)
        import ml_dtypes
        w28_t = (W2[e][:F_BLK] * 1024.0).astype(ml_dtypes.float8_e4m3)
        w28_t = w28_t.reshape(N_FC, 128, D).transpose(1, 0, 2)
        in_maps.append({
            "xeT": np.ascontiguousarray(xeT_t),
            "w1": np.ascontiguousarray(w1_t),
            "w2": np.ascontiguousarray(w2_t),
            "w28": np.ascontiguousarray(w28_t),
        })

    nc = _get_nc()
    res = run_bass_kernel_spmd(nc, in_maps, core_ids=list(range(E)))

    yf = np.zeros((T, D), dtype=np.float32)
    for e in range(E):
        ye = res.results[e]["y"].astype(np.float32).sum(axis=0)  # [CAP, D]
        m = valid[e]
        yf[dispatch[e][m]] = ye[m]
    return yf.reshape(B, S, D)


# revision 11
# speedup vs baseline: 1.1209x; 1.1209x over previous
"""MoE (top-1 routing, E=8 experts) Trainium2 kernel.

Strategy (expert-parallel across 8 NeuronCores):
  - Routing (softmax/argmax/capacity) is computed on host with jax-on-CPU,
    replicating the reference computation op-for-op so expert assignment
    matches bit-exactly.
  - Dispatch (the "all-to-all") happens host-side while building per-core
    inputs: core e receives the (<=2048) tokens routed to expert e, already
    gathered, scaled by gate probability, transposed to [D, cap], and cast
    to fp16 (same PE rate as fp32r, half the DMA bytes, FWL weight loads).
  - Each core runs Y_e = relu(Xe @ W1_e) @ W2_e as a dense FFN with all
    weights + tokens resident in SBUF. GEMM2 partials per F-block are
    evacuated as fp16 and summed on host (frees SBUF + vector engine,
    streams the output DMA throughout the kernel instead of a tail burst).
  - Combine: host sums the 8 F-block partials, scatters back to token order.
"""

import os
import sys

for _p in ("/opt/trn_rl_repo",):
    if os.path.isdir(_p) and _p not in sys.path:
        sys.path.insert(0, _p)

import numpy as np

B, S, D, F, E = 8, 2048, 1024, 4096, 8
T = B * S
CAP = T // E  # 2048, capacity_factor 1.0

F_BLK = 512          # F columns per outer block
N_FBLK = F // F_BLK  # 8
N_DC = D // 128      # 8 contraction chunks for GEMM1
N_FC = F_BLK // 128  # 4 contraction chunks for GEMM2 per block
N_TG = CAP // 128    # 16 token groups
N_TC = CAP // 512    # 4 token columns


def _build_nc():
    import concourse.bacc as bacc
    import concourse.mybir as mybir
    from concourse.bass import ds
    from concourse.tile import TileContext

    f32 = mybir.dt.float32
    f16 = mybir.dt.float16
    f8 = mybir.dt.float8e4

    nc = bacc.Bacc("TRN2", target_bir_lowering=False, debug=False, num_devices=E)

    # Host pre-tiles the inputs so every DMA line is >=4 KB contiguous per
    # partition: w1 [fo, p, dc, 512], w2 [fo, p, fc, 1024], xeT [c, p, dc, 256].
    xeT = nc.dram_tensor("xeT", [8, 128, N_DC, 256], f16, kind="ExternalInput")
    w1 = nc.dram_tensor("w1", [N_FBLK, 128, N_DC, F_BLK], f16, kind="ExternalInput")
    w2 = nc.dram_tensor("w2", [N_FBLK, 128, N_FC, D], f16, kind="ExternalInput")
    # fo=0's W2 block, scaled by 1024 and cast to e4m3 for DoubleRow matmuls.
    w28 = nc.dram_tensor("w28", [128, N_FC, D], f8, kind="ExternalInput")
    # Per-F-block GEMM2 partials; host sums over axis 0.
    y = nc.dram_tensor("y", [N_FBLK, CAP, D], f16, kind="ExternalOutput")

    x_r = xeT.ap().rearrange("c p dc j -> p c dc j")
    w1_r = w1.ap().rearrange("fo p dc j -> p fo dc j")
    w2_r = w2.ap().rearrange("fo p fc d -> p fo fc d")
    y_r = y.ap().rearrange("fo (tg p) d -> p fo tg d", p=128)

    with TileContext(nc) as tc:
        with (
            tc.tile_pool(name="sbuf", bufs=1) as sbuf,
            tc.tile_pool(name="wpool", bufs=3) as wpool,
            tc.tile_pool(name="spool", bufs=2) as spool,
            tc.tile_pool(name="psh", bufs=3, space="PSUM") as psh,
            tc.tile_pool(name="psy", bufs=5, space="PSUM") as psy,
        ):
            # PE warmup: dependency-light fp32 matmuls ramp the HAM clock to
            # 2.4 GHz while the first DMAs land. memset on the vector queue
            # starts earlier than gpsimd's.
            warm_sb = sbuf.tile([128, 384], f32, tag="warm")
            nc.vector.memset(warm_sb, 0)
            for _ in range(16):
                pwarm = psy.tile([128, 512], f32, tag="py")
                nc.tensor.matmul(
                    pwarm[:, :256], warm_sb[:, :128], warm_sb[:, ds(128, 256)],
                    start=True, stop=True,
                )

            # The gpsimd SWDGE queue moves data ~10x faster than the sync
            # HWDGE queue during the startup window, so everything the fo=0
            # block needs streams there, interleaved in consumption order:
            # w1[fo0], first xeT chunks, w2[fo0], remaining xeT chunks. The
            # bulk fo>=1 weights stream on sync in parallel (they are needed
            # only ~55us/block later).
            xeT_sb = sbuf.tile([128, N_DC, CAP], f16, tag="x")

            w28_sb = sbuf.tile([128, N_FC, D], f8, tag="w28")
            hT8 = sbuf.tile([128, N_FC, CAP], f8, tag="h8")

            # Weights stream through a 3-deep ring (wpool) so each block's
            # DMA is throttled by consumption two blocks earlier instead of
            # all 8 cores racing 16 MB each through HBM during startup.
            # Critical fo0 path: w1[fo0] on sync, xeT + fp8 w2[fo0] on gpsimd.
            w1_tiles = [None] * N_FBLK
            w2_tiles = [None] * N_FBLK

            def fetch_w1(fo, dc_split=False):
                t = wpool.tile([128, N_DC, F_BLK], f16, tag="w1")
                if dc_split:
                    for dc in range(N_DC):
                        nc.sync.dma_start(out=t[:, dc, :], in_=w1_r[:, fo, dc, :])
                else:
                    nc.sync.dma_start(out=t, in_=w1_r[:, fo, :, :])
                w1_tiles[fo] = t

            def fetch_w2(fo):
                t = wpool.tile([128, N_FC, D], f16, tag="w2")
                nc.sync.dma_start(out=t, in_=w2_r[:, fo, :, :])
                w2_tiles[fo] = t

            fetch_w1(0, dc_split=True)
            # xeT chunk 0 dc-split on gpsimd so the first GEMM1 chain can
            # pipeline with the DMA; remaining chunks alternate between the
            # two queues (each queue alone is slower than fo0's consumption).
            for dc in range(N_DC):
                nc.gpsimd.dma_start(
                    out=xeT_sb[:, dc, ds(0, 256)], in_=x_r[:, 0, dc, :]
                )
            nc.gpsimd.dma_start(out=w28_sb, in_=w28.ap())
            for tcix in range(1, 8):
                eng = nc.sync if tcix % 2 == 1 else nc.gpsimd
                eng.dma_start(
                    out=xeT_sb[:, :, ds(tcix * 256, 256)],
                    in_=x_r[:, tcix, :, :],
                )
            fetch_w1(1)
            fetch_w2(1)

            hT = sbuf.tile([128, N_FC, CAP], f16, tag="h")

            for fo in range(N_FBLK):
                # Prefetch weights two blocks ahead (ring depth 3).
                if fo + 2 < N_FBLK:
                    fetch_w1(fo + 2)
                    fetch_w2(fo + 2)
                w1t = w1_tiles[fo]
                # GEMM1: hT[f, t] = relu(sum_d W1[d, f] * XeT[d, t])
                # fo==0 walks 256-token columns to match streaming xeT
                # arrival; later blocks use full 512-wide moving operands.
                tok_cols = 2 * N_TC if fo == 0 else N_TC
                tok_w = CAP // tok_cols
                for tcix in range(tok_cols):
                    for fc in range(N_FC):
                        ph = psh.tile([128, 512], f32, tag="ph")
                        for dc in range(N_DC):
                            nc.tensor.matmul(
                                ph[:, :tok_w],
                                w1t[:, dc, ds(fc * 128, 128)],
                                xeT_sb[:, dc, ds(tcix * tok_w, tok_w)],
                                start=(dc == 0),
                                stop=(dc == N_DC - 1),
                            )
                        if fo == 0:
                            nc.scalar.activation(
                                hT8[:, fc, ds(tcix * tok_w, tok_w)],
                                ph[:, :tok_w],
                                mybir.ActivationFunctionType.Relu,
                                scale=16.0,
                            )
                        else:
                            nc.scalar.activation(
                                hT[:, fc, ds(tcix * tok_w, tok_w)],
                                ph[:, :tok_w],
                                mybir.ActivationFunctionType.Relu,
                            )

                # GEMM2: y_fo[t, d] = sum_f hT[f, t] * W2[f, d]; evacuate each
                # [128, 1024] token-group row as fp16 and stream it out on the
                # scalar (Activation) queue, summed across fo on host.
                for tg in range(N_TG):
                    if tg % 2 == 0:
                        stage = spool.tile([128, 2, D], f16, tag="st")
                    for dh in range(2):
                        py = psy.tile([128, 512], f32, tag="py")
                        if fo == 0:
                            # fp8 DoubleRow: contract f-chunk pairs (K=256 per
                            # matmul) at 2x rate; inputs carry a 16*1024 scale
                            # removed at evacuation.
                            for dq in range(2):
                                for pr in range(2):
                                    nc.tensor.matmul(
                                        py[:, ds(dq * 256, 256)],
                                        hT8[:, ds(2 * pr, 2), ds(tg * 128, 128)],
                                        w28_sb[:, ds(2 * pr, 2),
                                               ds(dh * 512 + dq * 256, 256)],
                                        start=(pr == 0),
                                        stop=(pr == 1),
                                        perf_mode=mybir.MatmulPerfMode.DoubleRow,
                                    )
                            nc.vector.tensor_scalar_mul(
                                stage[:, tg % 2, ds(dh * 512, 512)], py,
                                1.0 / 16384.0,
                            )
                        else:
                            for fc in range(N_FC):
                                nc.tensor.matmul(
                                    py,
                                    hT[:, fc, ds(tg * 128, 128)],
                                    w2_tiles[fo][:, fc, ds(dh * 512, 512)],
                                    start=(fc == 0),
                                    stop=(fc == N_FC - 1),
                                )
                            nc.vector.tensor_copy(stage[:, tg % 2, ds(dh * 512, 512)], py)
                    if tg % 2 == 1:
                        nc.scalar.dma_start(
                            out=y_r[:, fo, ds(tg - 1, 2), :],
                            in_=stage,
                        )

    nc.compile()
    return nc


_NC = None


def _get_nc():
    global _NC
    if _NC is None:
        _NC = _build_nc()
    return _NC


def _route(xf, Wr):
    """Replicates the reference routing (jax-on-CPU, op-for-op) so that
    expert assignment matches the fp32 reference bit-exactly."""
    try:
        import jax
        import jax.numpy as jnp

        cpu = jax.local_devices(backend="cpu")[0]
        with jax.default_device(cpu):
            xj = jnp.asarray(xf, dtype=jnp.float32)
            wj = jnp.asarray(Wr, dtype=jnp.float32)
            probs = jax.nn.softmax(xj @ wj, axis=-1)
            eidx_j = jnp.argmax(probs, axis=-1)
            p_tok_j = jnp.take_along_axis(probs, eidx_j[:, None], axis=1)[:, 0]
            eidx = np.asarray(eidx_j)
            p_tok = np.asarray(p_tok_j)
    except Exception:
        # numpy fallback (fp32, same math; argmax ties broken identically
        # by first-max)
        logits = xf.astype(np.float32) @ Wr.astype(np.float32)
        lmax = logits.max(axis=-1, keepdims=True)
        ex = np.exp(logits - lmax)
        probs = ex / ex.sum(axis=-1, keepdims=True)
        eidx = np.argmax(probs, axis=-1)
        p_tok = probs[np.arange(T), eidx]

    # Integer capacity logic (exact) in numpy.
    onehot = np.zeros((T, E), dtype=np.int64)
    onehot[np.arange(T), eidx] = 1
    rank = np.cumsum(onehot, axis=0) - onehot
    rank = rank[np.arange(T), eidx]  # earlier same-expert tokens
    keep = rank < CAP

    dispatch = np.zeros((E, CAP), dtype=np.int64)
    valid = np.zeros((E, CAP), dtype=bool)
    kept = np.nonzero(keep)[0]
    dispatch[eidx[kept], rank[kept]] = kept
    valid[eidx[kept], rank[kept]] = True
    return dispatch, valid, p_tok


def kernel(x, Wr, W1, W2):
    from concourse.bass_utils import run_bass_kernel_spmd

    x = np.asarray(x, dtype=np.float32)
    Wr = np.asarray(Wr, dtype=np.float32)
    W1 = np.asarray(W1, dtype=np.float32)
    W2 = np.asarray(W2, dtype=np.float32)

    xf = x.reshape(T, D)
    dispatch, valid, p_tok = _route(xf, Wr)

    in_maps = []
    for e in range(E):
        scale = np.where(valid[e], p_tok[dispatch[e]], 0.0).astype(np.float32)
        xe = xf[dispatch[e]] * scale[:, None]  # [CAP, D]; relu(s*x@W1)@W2 = s*y
        xeT_t = xe.T.astype(np.float16).reshape(N_DC, 128, 8, 256).transpose(2, 1, 0, 3)
        w1_t = W1[e].astype(np.float16).reshape(N_DC, 128, N_FBLK, F_BLK).transpose(2, 1, 0, 3)
        w2_t = W2[e].astype(np.float16).reshape(N_FBLK, N_FC, 128, D).transpose(0, 2, 1, 3)
        import ml_dtypes
        w28_t = (W2[e][:F_BLK] * 1024.0).astype(ml_dtypes.float8_e4m3)
        w28_t = w28_t.reshape(N_FC, 128, D).transpose(1, 0, 2)
        in_maps.append({
            "xeT": np.ascontiguousarray(xeT_t),
            "w1": np.ascontiguousarray(w1_t),
            "w2": np.ascontiguousarray(w2_t),
            "w28": np.ascontiguousarray(w28_t),
        })

    nc = _get_nc()
    res = run_bass_kernel_spmd(nc, in_maps, core_ids=list(range(E)))

    yf = np.zeros((T, D), dtype=np.float32)
    for e in range(E):
        ye = res.results[e]["y"].astype(np.float32).sum(axis=0)  # [CAP, D]
        m = valid[e]
        yf[dispatch[e][m]] = ye[m]
    return yf.reshape(B, S, D)
